# revision 1
# baseline (speedup 1.0000x reference)
"""TRN2 Bass kernel for nn_AutoRegressive (LSTM warmup + autoregressive decode).

Contract: kernel(**inputs) takes the FULL unsharded inputs
  inputs [2048, 48, 64], W [64, 4096], U [1024, 4096], b [4096],
  Wd [1024, 64], bd [64]
and returns the FULL output [2048, 64, 64] (float32), computed on 8
NeuronCores, data-parallel over the batch (256 rows per core).

Implementation notes:
- Transposed layout end-to-end: state hT [1024, 256] (units on partitions,
  batch on the free axis), so every matmul uses the weights in their natural
  layout as the stationary operand (out = lhsT.T @ rhs) and no on-chip
  transposes are needed anywhere.
- fp32r matmuls (1 cycle/row at N>=256, vs 4 for fp32). fp32r is fp32 with
  the low 12 mantissa bits zeroed; the multiply is exact given rounded
  inputs, accumulation is fp32 in PSUM. Inputs are pre-rounded on the host
  (RNE), on-chip producers (DVE/ACT) write fp32r directly.
- The bias b rides as row 64 of W_aug against a constant ones-row of the
  x operand; bd is applied by the DVE evacuation of the prediction.
- h is double-buffered across steps (z_t must read h_{t-1} while h_t is
  being written).
- Per step and unit, gate groups run in order i, f, g, o with PSUM banks
  [i,f | g] + [o], so the cell update (sigmoid/tanh on ACT, c/h on DVE)
  overlaps the later gate matmuls and the per-unit serial tail is short.
"""

import numpy as np

import concourse.mybir as mybir
import concourse.tile as tile
from concourse.bacc import Bacc
from concourse.bass_utils import run_bass_kernel_spmd

F32 = mybir.dt.float32
F32R = mybir.dt.float32r

B, T_IN, FEAT, UNITS, OUT_STEPS = 2048, 48, 64, 1024, 64
N_CORES = 8
BC = B // N_CORES  # 256
KT = UNITS // 128  # 8
GATE_N = 4 * UNITS  # 4096

SIG = mybir.ActivationFunctionType.Sigmoid
TANH = mybir.ActivationFunctionType.Tanh


def to_f32r(a: np.ndarray) -> np.ndarray:
    """Round fp32 to fp32r (11 explicit mantissa bits, RNE). Bit-matches HW."""
    u = np.ascontiguousarray(a, dtype=np.float32).view(np.uint32)
    r = (u + np.uint32(0x7FF) + ((u >> np.uint32(12)) & np.uint32(1))) & np.uint32(
        0xFFFFF000
    )
    return r.view(np.float32)


def build_lstm(n_warm: int = T_IN, n_dec: int = OUT_STEPS):
    """n_dec = number of outputs (first after warmup + n_dec-1 decode cells)."""
    nc = Bacc("TRN2", target_bir_lowering=False)
    xt_d = nc.dram_tensor("xt", [n_warm, 65, BC], F32R, kind="ExternalInput")
    U_d = nc.dram_tensor("U", [128, KT, 4 * KT, 128], F32R, kind="ExternalInput")
    W_d = nc.dram_tensor("W", [65, GATE_N], F32R, kind="ExternalInput")
    Wd_d = nc.dram_tensor("Wd", [128, KT, FEAT], F32R, kind="ExternalInput")
    bd_d = nc.dram_tensor("bd", [FEAT, 1], F32, kind="ExternalInput")
    out_d = nc.dram_tensor("outT", [n_dec, FEAT, BC], F32, kind="ExternalOutput")

    n_steps = n_warm + (n_dec - 1)

    with tile.TileContext(nc) as tc:
        with (
            tc.tile_pool(name="weights", bufs=1) as wp,
            tc.tile_pool(name="state", bufs=1) as sp,
            tc.tile_pool(name="xs", bufs=4) as xp,
            tc.tile_pool(name="acts", bufs=3) as ap,
            tc.tile_pool(name="scratch", bufs=2) as scp,
            tc.tile_pool(name="psA", bufs=2, space="PSUM") as psa_p,
            tc.tile_pool(name="psO", bufs=2, space="PSUM") as pso_p,
            tc.tile_pool(name="psP", bufs=1, space="PSUM") as psp_p,
        ):
            U_sb = wp.tile([128, KT, 4 * KT, 128], F32R)
            W_sb = wp.tile([65, GATE_N], F32R)
            Wd_sb = wp.tile([128, KT, FEAT], F32R)
            bd_sb = wp.tile([FEAT, 1], F32)
            nc.sync.dma_start(out=W_sb[:, :], in_=W_d[:, :])
            # two DMAs per unit chunk -> unit 0's weights land in ~half the
            # single-queue time, so step-0 matmuls start earlier
            for uu in range(KT):
                nc.sync.dma_start(out=U_sb[:, uu, 0 : 2 * KT, :], in_=U_d[:, uu, 0 : 2 * KT, :])
                nc.sync.dma_start(out=U_sb[:, uu, 2 * KT :, :], in_=U_d[:, uu, 2 * KT :, :])
            nc.sync.dma_start(out=Wd_sb[:, :, :], in_=Wd_d[:, :, :])
            nc.sync.dma_start(out=bd_sb[:, :], in_=bd_d[:, :])

            # h double-buffered across steps: matmuls read bank t%2, the
            # h-update writes bank (t+1)%2 (z must use h from the previous step)
            h_k = [
                [
                    sp.tile([128, BC], F32R, name=f"h{bk}_{k}", tag=f"h{bk}_{k}")
                    for k in range(KT)
                ]
                for bk in range(2)
            ]
            c_k = [sp.tile([128, BC], F32, name=f"c{k}", tag=f"c{k}") for k in range(KT)]
            xbuf = sp.tile([65, BC], F32R, name="xbuf", tag="xbuf")
            # memset can't write f32r; route zeros/ones through DVE adds
            zscr = sp.tile([128, BC], F32, name="zscr", tag="zscr")
            nc.vector.memset(zscr[:, :], 0.0)
            for k in range(KT):
                nc.vector.tensor_scalar_add(h_k[0][k][:, :], zscr[:, :], 0.0)
                nc.vector.memset(c_k[k][:, :], 0.0)
            nc.vector.tensor_scalar_add(xbuf[64:65, :], zscr[0:1, :], 1.0)
            nc.vector.tensor_scalar_add(xbuf[0:FEAT, :], zscr[0:FEAT, :], 0.0)

            for t in range(n_steps):
                h_rd = h_k[t % 2]
                h_wr = h_k[(t + 1) % 2]
                warm = t < n_warm
                if warm:
                    x_rhs = xp.tile([65, BC], F32R, tag="xstage")
                    nc.sync.dma_start(out=x_rhs[:, :], in_=xt_d[t, :, :])
                else:
                    x_rhs = xbuf

                for u in range(KT):
                    # psA [128, 768]: cols [0:512] = bank A (i, f),
                    # cols [512:768] = bank B (g). o goes to its own psO bank.
                    # Group order i, f, g, o: the c-update chain starts right
                    # after f (overlapping g/o matmuls), so the post-matmul
                    # tail per unit is just sigmoid(o) + h-mul (~1us).
                    psA = psa_p.tile([128, 3 * BC], F32, tag="psA")
                    psO = pso_p.tile([128, BC], F32, tag="psO")

                    def group(out_ap, gi):
                        for kt in range(KT):
                            nc.tensor.matmul(
                                out_ap,
                                lhsT=U_sb[:, u, gi * KT + kt, :],
                                rhs=h_rd[kt][:, :],
                                start=(kt == 0),
                                stop=False,
                            )
                        # W-matmul last: fp32r matmuls have a single wait slot,
                        # so the group's first matmul may only wait on h (DVE)
                        zoff = gi * UNITS + u * 128
                        nc.tensor.matmul(
                            out_ap,
                            lhsT=W_sb[:, zoff : zoff + 128],
                            rhs=x_rhs[:, :],
                            start=False,
                            stop=True,
                        )

                    group(psA[:, 0:BC], 0)  # i
                    group(psA[:, BC : 2 * BC], 1)  # f
                    actA = ap.tile([128, 3 * BC], F32, tag="actA")
                    # sigmoid(i,f) fires once bank A is complete, while PE
                    # streams the g/o groups
                    nc.scalar.activation(actA[:, 0 : 2 * BC], psA[:, 0 : 2 * BC], SIG)
                    tmp = scp.tile([128, BC], F32, tag="tmp")
                    # c = sig(f)*c  (overlaps g matmuls)
                    nc.vector.tensor_mul(
                        c_k[u][:, :], actA[:, BC : 2 * BC], c_k[u][:, :]
                    )

                    group(psA[:, 2 * BC :], 2)  # g
                    nc.scalar.activation(actA[:, 2 * BC :], psA[:, 2 * BC :], TANH)
                    # c += sig(i)*tanh(g); tanh(c)  (overlap o matmuls)
                    nc.vector.tensor_mul(tmp[:, :], actA[:, 0:BC], actA[:, 2 * BC :])
                    nc.vector.tensor_add(c_k[u][:, :], c_k[u][:, :], tmp[:, :])
                    tanc = scp.tile([128, BC], F32, tag="tanc")
                    nc.scalar.activation(tanc[:, :], c_k[u][:, :], TANH)

                    group(psO[:, :], 3)  # o
                    actO = ap.tile([128, BC], F32, tag="actO")
                    nc.scalar.activation(actO[:, :], psO[:, :], SIG)
                    # h = sig(o)*tanh(c), rounded to f32r on write
                    nc.vector.tensor_mul(h_wr[u][:, :], actO[:, :], tanc[:, :])

                # p-block: output prediction (also decode feedback)
                if t >= n_warm - 1:
                    psP = psp_p.tile([FEAT, BC], F32, tag="psP")
                    for kt in range(KT):
                        nc.tensor.matmul(
                            psP[:, :],
                            lhsT=Wd_sb[:, kt, :],
                            rhs=h_wr[kt][:, :],
                            start=(kt == 0),
                            stop=(kt == KT - 1),
                        )
                    nc.vector.tensor_scalar_add(
                        xbuf[0:FEAT, :], psP[:, :], bd_sb[:, :]
                    )
                    nc.sync.dma_start(
                        out=out_d[t - (n_warm - 1), :, :],
                        in_=xbuf[0:FEAT, :].bitcast(F32),
                    )
    nc.finalize()
    return nc


def prep_in_maps(inputs, W, U, b, Wd, bd, n_warm: int = T_IN):
    """Full unsharded reference inputs -> list of 8 per-core input maps."""
    W_aug = to_f32r(np.concatenate([W, b[None, :]], axis=0))  # [65, 4096]
    # U[kt*128+p, gi*1024+u*128+c] -> [p, u, gi*KT+kt, c] (per-unit DMA chunks)
    U_l = to_f32r(
        np.ascontiguousarray(
            U.reshape(KT, 128, 4, KT, 128)
            .transpose(1, 3, 2, 0, 4)
            .reshape(128, KT, 4 * KT, 128)
        )
    )
    Wd_l = to_f32r(Wd.reshape(KT, 128, FEAT).transpose(1, 0, 2))  # [128, KT, 64]
    bd_l = np.ascontiguousarray(bd.astype(np.float32)[:, None])  # [64, 1]

    in_maps = []
    for c in range(N_CORES):
        xc = inputs[c * BC : (c + 1) * BC, :n_warm, :]  # [BC, T, F]
        xt = xc.transpose(1, 2, 0)  # [T, F, BC]
        xt_aug = np.concatenate(
            [xt, np.ones((n_warm, 1, BC), np.float32)], axis=1
        )  # [T, 65, BC]
        in_maps.append(
            {
                "xt": np.ascontiguousarray(to_f32r(xt_aug)),
                "U": U_l,
                "W": W_aug,
                "Wd": Wd_l,
                "bd": bd_l,
            }
        )
    return in_maps


def assemble_output(results, n_dec: int = OUT_STEPS):
    """Per-core outT [n_dec, 64, BC] -> full [B, n_dec, 64]."""
    outs = []
    for c in range(N_CORES):
        o = results[c]["outT"]  # [n_dec, FEAT, BC]
        outs.append(o.transpose(2, 0, 1))  # [BC, n_dec, FEAT]
    return np.ascontiguousarray(np.concatenate(outs, axis=0).astype(np.float32))


_NC_CACHE = {}


def kernel(inputs, W, U, b, Wd, bd):
    inputs = np.asarray(inputs, dtype=np.float32)
    W = np.asarray(W, dtype=np.float32)
    U = np.asarray(U, dtype=np.float32)
    b = np.asarray(b, dtype=np.float32)
    Wd = np.asarray(Wd, dtype=np.float32)
    bd = np.asarray(bd, dtype=np.float32)
    assert inputs.shape == (B, T_IN, FEAT), inputs.shape

    if "nc" not in _NC_CACHE:
        _NC_CACHE["nc"] = build_lstm(T_IN, OUT_STEPS)
    nc = _NC_CACHE["nc"]

    in_maps = prep_in_maps(inputs, W, U, b, Wd, bd)
    res = run_bass_kernel_spmd(nc, in_maps, core_ids=list(range(N_CORES)))
    return assemble_output(res.results)

